# revision 16
# baseline (speedup 1.0000x reference)
"""GNN message passing (gather + segment-sum) on 8 TRN2 NeuronCores.

Strategy (edge-parallel with node-partitioned output; no collectives):
  - Host: bucket edges by (core = dst // 6250, src-half, dst-window-of-128).
    Core c owns output rows [c*6250, (c+1)*6250) so partial sums ARE final --
    no all-reduce needed.  Within a core, edges are grouped by 128-node dst
    windows; each group is padded to a multiple of 128 edges (common tile
    counts across all 8 cores so one SPMD program serves every core).
  - Device, per core:
      * bulk `dma_gather` of x[src] rows (fp16 table, 256B/row) from HBM into
        SBUF, in 1024-index chunks (Q7 SWDGE gather; int16 indices, so the
        table is addressed as two halves: rows [0,32768) and [32768,50000)).
        Calls round-robin over 4 SWDGE queues: each queue has its own
        descriptor ring/carveout and Q7 core pair, so queue k+1's descriptor
        generation overlaps queue k's DMA drain + ring reclaim (the single-
        queue version serializes on reclaim and runs ~2x slower).
      * per 128-edge tile, build one-hot S[e, n] = (dst_local[e] == n) on the
        DVE with a broadcast `is_equal` against an iota row constant, batched
        8 tiles per instruction via stride-0 3D access patterns.
      * matmul S^T @ G accumulated in PSUM per 128-node window: the PE does
        the segment reduction.  PSUM (f32) -> SBUF accumulator -> HBM out.
  - Host: concatenate the 8 per-core [6250, 128] slices.

The one-hot/matmul trick makes the scatter-add race-free.  The critical path
is the SWDGE gather pipeline at ~2.3 ns/row: per-call Q7 desc-gen itself is
cheap (~0.1-0.9us engine slices), but the descriptor ring holds only ONE
in-flight call per (queue, direction) -- runtime-fixed carveout, NOT scaled by
dynamic_dma_scratch_size -- so each call's decode stalls in await_space until
the previous same-queue call's DMA fully drains.  Each 4-queue rotation of
1024-idx calls costs ~9.1us: 4 x ~1.7us drains (256B/descriptor random HBM
reads, 16 engines, ~25.8ns/desc/engine) + ~2.4us sem-propagation/reclaim.
Mid-kernel the Pool engine (awaits included) is ~99% occupied; DMA engines
~50%, DVE ~57% (is_equal), PE ~42% (ldweights+matmul pipeline at 106ns/tile).
Call-size sweep: 8 tiles 238us / 7 t 243us / 6 t 292us / 4 t 262us; 9+ tiles
(1152+ idx) wedges the core (Q7 ucode cap).  Startup is ~15us (engine
preamble + chunked idx head load + warmup gather that pre-warms the Q7
gather-kernel icache) and the drain/compute/teardown tail is ~19us.
Gather-call pipelining needs gpool bufs >= ~6: with fewer buffers the next
call stalls on G-tile reuse and the queues cannot overlap (3 bufs -> ~445us).
"""

import os
import numpy as np

N = 50000          # nodes
D = 128            # feature dim
C = 8              # cores
E_TOT = 640000     # edges (any count works; hardcoded shapes only use N, D)
NLOC = N // C      # 6250 output rows per core
P = 128
N_WIN = (NLOC + P - 1) // P        # 49 windows of 128 dst nodes per core
NLOC_PAD = N_WIN * P               # 6272 (padded output rows per core)
SPLIT = 32768                      # int16 gather-index limit
SENT = 300.0                       # dst sentinel for padded edges (never matches iota 0..127)
CHUNK_TILES = 8                    # 128-edge tiles per dma_gather call (1024 idx).
                                   # Measured call-size sweep (HW exec): 8 tiles
                                   # 238us / 7 tiles 243us / 6 tiles 292us /
                                   # 4 tiles 262us; >8 tiles (1152+ idx) wedges
                                   # the core (Q7 gather-ucode cap).  The pace
                                   # is set by the SWDGE ring: one in-flight
                                   # call per (queue, direction) ring, so each
                                   # 4-queue rotation costs its 4 DMA drains
                                   # plus ~2.4us of sem-propagation/reclaim --
                                   # bigger calls amortize that fixed cost
                                   # best.  A 4x dynamic_dma_scratch_size does
                                   # NOT deepen the ring (runtime-fixed).

LAST_RESULT = None                 # BassKernelResults of the most recent run (for test.py)

_prog_cache = {}


def _ensure_ntff_hook():
    """Provide antenv.axon_hooks (missing from this image) so
    run_bass_kernel_spmd(trace=True) under axon can capture NTFF profiles.
    Harmless no-op when tracing is off or pieces are unavailable."""
    import sys
    import types
    try:
        import antenv.axon_hooks  # noqa: F401
        return
    except ImportError:
        pass
    try:
        import antenv
        mod = types.ModuleType("antenv.axon_hooks")
        mod._hook = None
        mod.set_axon_ntff_profile_hook = lambda h: setattr(mod, "_hook", h)
        mod.get_axon_ntff_profile_hook = lambda: mod._hook
        sys.modules["antenv.axon_hooks"] = mod
        antenv.axon_hooks = mod
        from trn_agent_boot.trn_boot import _ntff_profile_via_ctypes
        so_path = "/opt/axon/libaxon_pjrt.so"
        if os.path.exists(so_path):
            mod.set_axon_ntff_profile_hook(_ntff_profile_via_ctypes(so_path))
    except Exception:
        pass


def _host_prep(x, edge_index):
    """Bucket + pad edges; build per-core device input arrays."""
    x = np.asarray(x, dtype=np.float32)
    ei = np.asarray(edge_index)
    src = ei[0].astype(np.int64)
    dst = ei[1].astype(np.int64)
    E = src.shape[0]

    core = dst // NLOC
    dloc = dst - core * NLOC
    win = dloc >> 7                 # dst window within core
    pcol = dloc & 127               # dst node within window
    half = (src >= SPLIT).astype(np.int64)

    # counts[c, h, w]
    counts = np.zeros((C, 2, N_WIN), np.int64)
    np.add.at(counts, (core, half, win), 1)
    # common (max-over-cores) tile counts so one SPMD program fits all cores
    T = (-(-counts // P)).max(axis=0)        # [2, N_WIN] tiles per (half, window)
    T[0] = np.maximum(T[0], 1)               # lo pass initializes every window's acc

    L = T.sum(axis=1) * P                    # padded edges per half
    tile_base = np.zeros((2, N_WIN), np.int64)
    tile_base[0, 1:] = np.cumsum(T[0])[:-1]
    tile_base[1, 1:] = np.cumsum(T[1])[:-1]

    # sort edges by (core, half, window, src).  The src sub-sort is a pure
    # DMA-locality optimization: the Q7 swizzle hands each DMA engine
    # consecutive QUADS of gather slots, so sorted srcs give each engine
    # runs of near-adjacent 256B table rows (HBM row-buffer hits) instead of
    # uniformly random addresses.  The await_space release tracks the
    # SLOWEST of the 16 engines, so cutting drain variance shortens every
    # 4-call rotation.
    order = np.lexsort((src, win, half, core))
    s_src = src[order]
    s_p = pcol[order]
    gsz = counts.reshape(-1)
    gstart = np.zeros(C * 2 * N_WIN + 1, np.int64)
    np.cumsum(gsz, out=gstart[1:])

    xh = np.ascontiguousarray(x.astype(np.float16))
    iota = np.tile(np.arange(P, dtype=np.float16)[None, :], (P, 1))

    def wrap_idx(a):  # int16 [L] -> [128, L//16] (16-part wrap, replicated x8)
        w16 = np.ascontiguousarray(a.reshape(-1, 16).T)
        return np.ascontiguousarray(np.tile(w16, (8, 1)))

    per_core = []
    for c in range(C):
        srcs = [np.zeros(L[0], np.int16), np.zeros(L[1], np.int16)]
        dstp = [np.full(L[0], SENT, np.float16), np.full(L[1], SENT, np.float16)]
        for h in range(2):
            for w in range(N_WIN):
                g = (c * 2 + h) * N_WIN + w
                a, b = gstart[g], gstart[g + 1]
                n = b - a
                if n == 0:
                    continue
                pos = tile_base[h, w] * P
                adj = 0 if h == 0 else SPLIT
                srcs[h][pos:pos + n] = (s_src[a:b] - adj).astype(np.int16)
                dstp[h][pos:pos + n] = s_p[a:b].astype(np.float16)
        dstp_all = np.concatenate(dstp)                       # [L0 + L1]
        dstp_tile = np.ascontiguousarray(dstp_all.reshape(-1, P).T)  # [128, T_tot]
        meta = np.concatenate([dstp_tile, iota], axis=1)      # [128, T_tot + 128]
        m = {
            "xh": xh,
            "idx_lo": wrap_idx(srcs[0]),
            "idx_hi": wrap_idx(srcs[1]),
            "meta": np.ascontiguousarray(meta),
        }
        per_core.append(m)

    return per_core, tuple(T[0]), tuple(T[1]), int(L[0]), int(L[1])


def _build_program(T_lo, T_hi, L_lo, L_hi):
    import concourse.bass as bass
    import concourse.tile as tile
    import concourse.mybir as mybir
    from concourse import bacc

    dt = mybir.dt
    nc = bacc.Bacc("TRN2", target_bir_lowering=False, debug=False, num_devices=C,
                   num_swdge_queues=4)

    xh = nc.dram_tensor("xh", [N, D], dt.float16, kind="ExternalInput")
    L_tot = L_lo + L_hi
    HEADC = min(CHUNK_TILES * 8, L_lo // 16)
    idxlo_d = nc.dram_tensor("idx_lo", [128, L_lo // 16], dt.int16, kind="ExternalInput")
    idxhi_d = nc.dram_tensor("idx_hi", [128, L_hi // 16], dt.int16, kind="ExternalInput")
    T_tot = L_tot // P
    meta_d = nc.dram_tensor("meta", [128, T_tot + 128], dt.float16, kind="ExternalInput")
    out_d = nc.dram_tensor("out", [NLOC_PAD, D], dt.float32, kind="ExternalOutput")

    with tile.TileContext(nc) as tc:
        with (
            tc.tile_pool(name="metap", bufs=1) as metap,
            tc.tile_pool(name="gp", bufs=10) as gpool,
            tc.tile_pool(name="sp", bufs=10) as spool,
            tc.tile_pool(name="pp", bufs=6, space="PSUM") as ppool,
            tc.tile_pool(name="accp", bufs=1) as accp,
        ):
            # Warm the Q7 gather-kernel icache off the critical path: a tiny
            # 128-idx gather of row 0 (zeroed indices) issued right after the
            # engine preamble, while the real index tables are still loading.
            # The first real call then runs at steady-state cost (~2.4us)
            # instead of paying the ~6us cold-start.
            warm_idx = metap.tile([128, 8], dt.int16, tag="warmidx", name="warm_idx")
            nc.gpsimd.memset(warm_idx[:], 0)
            warm_g = metap.tile([128, 128], dt.float16, tag="warmg", name="warm_g")

            # Chunked idx_lo load: gather call c only depends on its own
            # column slice, so the first call can start as soon as the 16KB
            # head chunk lands instead of waiting for the full 0.9MB table.
            idxlo_t = metap.tile([128, L_lo // 16], dt.int16, tag="idxlo", name="idxlo_t")
            CUTS = [0, HEADC]
            for cut in (512, 1536):
                if HEADC < cut < L_lo // 16:
                    CUTS.append(cut)
            CUTS.append(L_lo // 16)
            for ci in range(len(CUTS) - 1):
                a, b = CUTS[ci], CUTS[ci + 1]
                eng = nc.sync if ci % 2 == 0 else nc.scalar
                eng.dma_start(idxlo_t[:, a:b], idxlo_d[:, a:b])
            idxhi_t = metap.tile([128, L_hi // 16], dt.int16, tag="idxhi", name="idxhi_t")
            nc.scalar.dma_start(idxhi_t[:], idxhi_d[:])
            meta_t = metap.tile([128, T_tot + 128], dt.float16, tag="meta", name="meta_t")
            nc.scalar.dma_start(meta_t[:], meta_d[:])

            nc.gpsimd.dma_gather(
                warm_g[:].rearrange("p (t f) -> p t f", f=128),
                xh[:SPLIT],
                warm_idx[:],
                128,
                nc.gpsimd.to_reg(128),
                D,
                queue_num=0,
            )
            dstp_t = meta_t[:, :T_tot]
            iota_t = meta_t[:, T_tot:]
            iota3 = iota_t.rearrange("p (a f) -> p a f", a=1)

            acc = accp.tile([128, N_WIN * P], dt.float32, tag="acc")

            # hoist num_idxs registers: one MOVE per distinct call size instead
            # of one per call (removes per-call WAR hazards on the reg)
            _nreg = {}

            def nreg(n):
                if n not in _nreg:
                    _nreg[n] = nc.gpsimd.to_reg(n)
                return _nreg[n]

            gt = 0  # global tile index (column into dstp_t)
            S4 = None
            for h in range(2):
                Th = T_lo if h == 0 else T_hi
                total_tiles = sum(Th)
                if total_tiles == 0:
                    continue
                src_view = xh[:SPLIT] if h == 0 else xh[SPLIT:]
                idxh_t = idxlo_t if h == 0 else idxhi_t
                th = 0        # tile index within this half
                G = None
                ntc = 0       # tiles in current chunk
                for wi in range(N_WIN):
                    tw = Th[wi]
                    if tw == 0:
                        continue
                    pt = ppool.tile([128, 128], dt.float32, tag="psum")
                    for t in range(tw):
                        cslot = th % CHUNK_TILES
                        if cslot == 0:
                            ntc = min(CHUNK_TILES, total_tiles - th)
                            G = gpool.tile([128, ntc * 128], dt.float16, tag="gather")
                            nidx = ntc * 128
                            nc.gpsimd.dma_gather(
                                G[:].rearrange("p (t f) -> p t f", f=128),
                                src_view,
                                idxh_t[:, th * 8:(th + ntc) * 8],
                                nidx,
                                nreg(nidx),
                                D,
                                queue_num=(th // CHUNK_TILES) % 4,
                            )
                        if gt % 8 == 0:
                            nb = min(8, T_tot - gt)
                            S4 = spool.tile([128, nb, 128], dt.float16, tag="sel")
                            nc.vector.tensor_tensor(
                                out=S4[:],
                                in0=dstp_t[:, gt:gt + nb].to_broadcast([128, nb, 128]),
                                in1=iota3.to_broadcast([128, nb, 128]),
                                op=mybir.AluOpType.is_equal,
                            )
                        nc.tensor.matmul(
                            pt[:],
                            S4[:, gt % 8, :],
                            G[:, cslot * 128:(cslot + 1) * 128],
                            start=(t == 0),
                            stop=(t == tw - 1),
                        )
                        th += 1
                        gt += 1
                    lo, hi = wi * 128, (wi + 1) * 128
                    if h == 0:
                        nc.vector.tensor_copy(acc[:, lo:hi], pt[:])
                    else:
                        nc.vector.tensor_add(acc[:, lo:hi], acc[:, lo:hi], pt[:])
                    last_touch = (h == 1) or (T_hi[wi] == 0)
                    if last_touch:
                        nc.sync.dma_start(out_d[lo:hi, :], acc[:, lo:hi])
    nc.compile()
    return nc


def kernel(x, edge_index):
    global LAST_RESULT
    _ensure_ntff_hook()
    from concourse.bass_utils import run_bass_kernel_spmd

    per_core, T_lo, T_hi, L_lo, L_hi = _host_prep(x, edge_index)

    key = (T_lo, T_hi)
    if key not in _prog_cache:
        _prog_cache[key] = _build_program(T_lo, T_hi, L_lo, L_hi)
    nc = _prog_cache[key]

    res = run_bass_kernel_spmd(nc, per_core, core_ids=list(range(C)))
    LAST_RESULT = res
    out = np.concatenate([r["out"][:NLOC] for r in res.results], axis=0)
    return out.astype(np.float32)



# revision 18
# speedup vs baseline: 1.0276x; 1.0276x over previous
"""GNN message passing (gather + segment-sum) on 8 TRN2 NeuronCores.

Strategy (edge-parallel with node-partitioned output; no collectives):
  - Host: bucket edges by (core = dst // 6250, src-half, dst-window-of-128).
    Core c owns output rows [c*6250, (c+1)*6250) so partial sums ARE final --
    no all-reduce needed.  Within a core, edges are grouped by 128-node dst
    windows; each group is padded to a multiple of 128 edges (common tile
    counts across all 8 cores so one SPMD program serves every core).
  - Device, per core:
      * bulk `dma_gather` of x[src] rows (fp16 table, 256B/row) from HBM into
        SBUF, in 1024-index chunks (Q7 SWDGE gather; int16 indices, so the
        table is addressed as two halves: rows [0,32768) and [32768,50000)).
        Calls round-robin over 4 SWDGE queues: each queue has its own
        descriptor ring/carveout and Q7 core pair, so queue k+1's descriptor
        generation overlaps queue k's DMA drain + ring reclaim (the single-
        queue version serializes on reclaim and runs ~2x slower).
      * per 128-edge tile, build one-hot S[e, n] = (dst_local[e] == n) on the
        DVE with a broadcast `is_equal` against an iota row constant, batched
        8 tiles per instruction via stride-0 3D access patterns.
      * matmul S^T @ G accumulated in PSUM per 128-node window: the PE does
        the segment reduction.  PSUM (f32) -> SBUF accumulator -> HBM out.
  - Host: concatenate the 8 per-core [6250, 128] slices.

The one-hot/matmul trick makes the scatter-add race-free.  The critical path
is the SWDGE gather pipeline at ~2.3 ns/row: per-call Q7 desc-gen itself is
cheap (~0.1-0.9us engine slices), but the descriptor ring holds only ONE
in-flight call per (queue, direction) -- runtime-fixed carveout, NOT scaled by
dynamic_dma_scratch_size -- so each call's decode stalls in await_space until
the previous same-queue call's DMA fully drains.  Each 4-queue rotation of
1024-idx calls costs ~9.1us: 4 x ~1.7us drains (256B/descriptor random HBM
reads, 16 engines, ~25.8ns/desc/engine) + ~2.4us sem-propagation/reclaim.
Mid-kernel the Pool engine (awaits included) is ~99% occupied; DMA engines
~50%, DVE ~57% (is_equal), PE ~42% (ldweights+matmul pipeline at 106ns/tile).
Call-size sweep: 8 tiles 238us / 7 t 243us / 6 t 292us / 4 t 262us; 9+ tiles
(1152+ idx) wedges the core (Q7 ucode cap).  Startup is ~15us (engine
preamble + chunked idx head load + warmup gather that pre-warms the Q7
gather-kernel icache) and the drain/compute/teardown tail is ~19us.
Gather-call pipelining needs gpool bufs >= ~6: with fewer buffers the next
call stalls on G-tile reuse and the queues cannot overlap (3 bufs -> ~445us).
"""

import os
import numpy as np

N = 50000          # nodes
D = 128            # feature dim
C = 8              # cores
E_TOT = 640000     # edges (any count works; hardcoded shapes only use N, D)
NLOC = N // C      # 6250 output rows per core
P = 128
N_WIN = (NLOC + P - 1) // P        # 49 windows of 128 dst nodes per core
NLOC_PAD = N_WIN * P               # 6272 (padded output rows per core)
SPLIT = 32768                      # int16 gather-index limit
SENT = 300.0                       # dst sentinel for padded edges (never matches iota 0..127)
CHUNK_TILES = 8                    # 128-edge tiles per dma_gather call (1024 idx).
                                   # Measured call-size sweep (HW exec): 8 tiles
                                   # 238us / 7 tiles 243us / 6 tiles 292us /
                                   # 4 tiles 262us; >8 tiles (1152+ idx) wedges
                                   # the core (Q7 gather-ucode cap).  The pace
                                   # is set by the SWDGE ring: one in-flight
                                   # call per (queue, direction) ring, so each
                                   # 4-queue rotation costs its 4 DMA drains
                                   # plus ~2.4us of sem-propagation/reclaim --
                                   # bigger calls amortize that fixed cost
                                   # best.  A 4x dynamic_dma_scratch_size does
                                   # NOT deepen the ring (runtime-fixed).

LAST_RESULT = None                 # BassKernelResults of the most recent run (for test.py)

_prog_cache = {}


def _ensure_ntff_hook():
    """Provide antenv.axon_hooks (missing from this image) so
    run_bass_kernel_spmd(trace=True) under axon can capture NTFF profiles.
    Harmless no-op when tracing is off or pieces are unavailable."""
    import sys
    import types
    try:
        import antenv.axon_hooks  # noqa: F401
        return
    except ImportError:
        pass
    try:
        import antenv
        mod = types.ModuleType("antenv.axon_hooks")
        mod._hook = None
        mod.set_axon_ntff_profile_hook = lambda h: setattr(mod, "_hook", h)
        mod.get_axon_ntff_profile_hook = lambda: mod._hook
        sys.modules["antenv.axon_hooks"] = mod
        antenv.axon_hooks = mod
        from trn_agent_boot.trn_boot import _ntff_profile_via_ctypes
        so_path = "/opt/axon/libaxon_pjrt.so"
        if os.path.exists(so_path):
            mod.set_axon_ntff_profile_hook(_ntff_profile_via_ctypes(so_path))
    except Exception:
        pass


def _host_prep(x, edge_index):
    """Bucket + pad edges; build per-core device input arrays."""
    x = np.asarray(x, dtype=np.float32)
    ei = np.asarray(edge_index)
    src = ei[0].astype(np.int64)
    dst = ei[1].astype(np.int64)
    E = src.shape[0]

    core = dst // NLOC
    dloc = dst - core * NLOC
    win = dloc >> 7                 # dst window within core
    pcol = dloc & 127               # dst node within window
    half = (src >= SPLIT).astype(np.int64)

    # counts[c, h, w]
    counts = np.zeros((C, 2, N_WIN), np.int64)
    np.add.at(counts, (core, half, win), 1)
    # common (max-over-cores) tile counts so one SPMD program fits all cores
    T = (-(-counts // P)).max(axis=0)        # [2, N_WIN] tiles per (half, window)
    T[0] = np.maximum(T[0], 1)               # lo pass initializes every window's acc

    L = T.sum(axis=1) * P                    # padded edges per half
    tile_base = np.zeros((2, N_WIN), np.int64)
    tile_base[0, 1:] = np.cumsum(T[0])[:-1]
    tile_base[1, 1:] = np.cumsum(T[1])[:-1]

    # sort edges by (core, half, window, src).  The src sub-sort is a pure
    # DMA-locality optimization: the Q7 swizzle hands each DMA engine
    # consecutive QUADS of gather slots, so sorted srcs give each engine
    # runs of near-adjacent 256B table rows (HBM row-buffer hits) instead of
    # uniformly random addresses.  The await_space release tracks the
    # SLOWEST of the 16 engines, so cutting drain variance shortens every
    # 4-call rotation.
    order = np.lexsort((src, win, half, core))
    s_src = src[order]
    s_p = pcol[order]
    gsz = counts.reshape(-1)
    gstart = np.zeros(C * 2 * N_WIN + 1, np.int64)
    np.cumsum(gsz, out=gstart[1:])

    xh = np.ascontiguousarray(x.astype(np.float16))
    iota = np.tile(np.arange(P, dtype=np.float16)[None, :], (P, 1))

    def wrap_idx(a):  # int16 [L] -> [128, L//16] (16-part wrap, replicated x8)
        w16 = np.ascontiguousarray(a.reshape(-1, 16).T)
        return np.ascontiguousarray(np.tile(w16, (8, 1)))

    per_core = []
    for c in range(C):
        srcs = [np.zeros(L[0], np.int16), np.zeros(L[1], np.int16)]
        dstp = [np.full(L[0], SENT, np.float16), np.full(L[1], SENT, np.float16)]
        for h in range(2):
            for w in range(N_WIN):
                g = (c * 2 + h) * N_WIN + w
                a, b = gstart[g], gstart[g + 1]
                n = b - a
                if n == 0:
                    continue
                pos = tile_base[h, w] * P
                adj = 0 if h == 0 else SPLIT
                srcs[h][pos:pos + n] = (s_src[a:b] - adj).astype(np.int16)
                dstp[h][pos:pos + n] = s_p[a:b].astype(np.float16)
        dstp_all = np.concatenate(dstp)                       # [L0 + L1]
        dstp_tile = np.ascontiguousarray(dstp_all.reshape(-1, P).T)  # [128, T_tot]
        meta = np.concatenate([dstp_tile, iota], axis=1)      # [128, T_tot + 128]
        m = {
            "xh": xh,
            "idx_lo": wrap_idx(srcs[0]),
            "idx_hi": wrap_idx(srcs[1]),
            "meta": np.ascontiguousarray(meta),
        }
        per_core.append(m)

    return per_core, tuple(T[0]), tuple(T[1]), int(L[0]), int(L[1])


def _build_program(T_lo, T_hi, L_lo, L_hi):
    import concourse.bass as bass
    import concourse.tile as tile
    import concourse.mybir as mybir
    from concourse import bacc

    dt = mybir.dt
    nc = bacc.Bacc("TRN2", target_bir_lowering=False, debug=False, num_devices=C,
                   num_swdge_queues=4)

    xh = nc.dram_tensor("xh", [N, D], dt.float16, kind="ExternalInput")
    L_tot = L_lo + L_hi
    HEADC = min(CHUNK_TILES * 8, L_lo // 16)
    idxlo_d = nc.dram_tensor("idx_lo", [128, L_lo // 16], dt.int16, kind="ExternalInput")
    idxhi_d = nc.dram_tensor("idx_hi", [128, L_hi // 16], dt.int16, kind="ExternalInput")
    T_tot = L_tot // P
    meta_d = nc.dram_tensor("meta", [128, T_tot + 128], dt.float16, kind="ExternalInput")
    out_d = nc.dram_tensor("out", [NLOC_PAD, D], dt.float32, kind="ExternalOutput")

    with tile.TileContext(nc) as tc:
        with (
            tc.tile_pool(name="metap", bufs=1) as metap,
            tc.tile_pool(name="gp", bufs=10) as gpool,
            tc.tile_pool(name="sp", bufs=10) as spool,
            tc.tile_pool(name="pp", bufs=6, space="PSUM") as ppool,
            tc.tile_pool(name="accp", bufs=1) as accp,
        ):
            # Warm the Q7 gather-kernel icache off the critical path: a tiny
            # 128-idx gather of row 0 (zeroed indices) issued right after the
            # engine preamble, while the real index tables are still loading.
            # The first real call then runs at steady-state cost (~2.4us)
            # instead of paying the ~6us cold-start.
            warm_idx = metap.tile([128, 8], dt.int16, tag="warmidx", name="warm_idx")
            nc.gpsimd.memset(warm_idx[:], 0)
            warm_g = metap.tile([128, 128], dt.float16, tag="warmg", name="warm_g")

            # Chunked idx_lo load: gather call c only depends on its own
            # column slice, so the first call can start as soon as the 16KB
            # head chunk lands instead of waiting for the full 0.9MB table.
            idxlo_t = metap.tile([128, L_lo // 16], dt.int16, tag="idxlo", name="idxlo_t")
            CUTS = [0, HEADC]
            for cut in (512, 1536):
                if HEADC < cut < L_lo // 16:
                    CUTS.append(cut)
            CUTS.append(L_lo // 16)
            for ci in range(len(CUTS) - 1):
                a, b = CUTS[ci], CUTS[ci + 1]
                eng = nc.sync if ci % 2 == 0 else nc.scalar
                eng.dma_start(idxlo_t[:, a:b], idxlo_d[:, a:b])
            idxhi_t = metap.tile([128, L_hi // 16], dt.int16, tag="idxhi", name="idxhi_t")
            nc.scalar.dma_start(idxhi_t[:], idxhi_d[:])
            meta_t = metap.tile([128, T_tot + 128], dt.float16, tag="meta", name="meta_t")
            nc.scalar.dma_start(meta_t[:], meta_d[:])

            nc.gpsimd.dma_gather(
                warm_g[:].rearrange("p (t f) -> p t f", f=128),
                xh[:SPLIT],
                warm_idx[:],
                128,
                nc.gpsimd.to_reg(128),
                D,
                queue_num=3,
            )
            dstp_t = meta_t[:, :T_tot]
            iota_t = meta_t[:, T_tot:]
            iota3 = iota_t.rearrange("p (a f) -> p a f", a=1)

            acc = accp.tile([128, N_WIN * P], dt.float32, tag="acc")

            # hoist num_idxs registers: one MOVE per distinct call size instead
            # of one per call (removes per-call WAR hazards on the reg)
            _nreg = {}

            def nreg(n):
                if n not in _nreg:
                    _nreg[n] = nc.gpsimd.to_reg(n)
                return _nreg[n]

            gt = 0  # global tile index (column into dstp_t)
            S4 = None
            for h in range(2):
                Th = T_lo if h == 0 else T_hi
                total_tiles = sum(Th)
                if total_tiles == 0:
                    continue
                src_view = xh[:SPLIT] if h == 0 else xh[SPLIT:]
                idxh_t = idxlo_t if h == 0 else idxhi_t
                th = 0        # tile index within this half
                G = None
                ntc = 0       # tiles in current chunk
                for wi in range(N_WIN):
                    tw = Th[wi]
                    if tw == 0:
                        continue
                    pt = ppool.tile([128, 128], dt.float32, tag="psum")
                    for t in range(tw):
                        cslot = th % CHUNK_TILES
                        if cslot == 0:
                            ntc = min(CHUNK_TILES, total_tiles - th)
                            G = gpool.tile([128, ntc * 128], dt.float16, tag="gather")
                            nidx = ntc * 128
                            nc.gpsimd.dma_gather(
                                G[:].rearrange("p (t f) -> p t f", f=128),
                                src_view,
                                idxh_t[:, th * 8:(th + ntc) * 8],
                                nidx,
                                nreg(nidx),
                                D,
                                # Reversed rotation (q3,q2,q1,q0): the LAST
                                # call of each rotation gates the pipeline
                                # (its await blocks the next rotation's
                                # issue), and the DMA engines service queue
                                # rings in index-priority order -- so put the
                                # gating call on queue 0 (highest priority)
                                # so its drain is not starved behind the
                                # other three queues' newer descriptors.
                                queue_num=3 - (th // CHUNK_TILES) % 4,
                            )
                        if gt % 8 == 0:
                            nb = min(8, T_tot - gt)
                            S4 = spool.tile([128, nb, 128], dt.float16, tag="sel")
                            nc.vector.tensor_tensor(
                                out=S4[:],
                                in0=dstp_t[:, gt:gt + nb].to_broadcast([128, nb, 128]),
                                in1=iota3.to_broadcast([128, nb, 128]),
                                op=mybir.AluOpType.is_equal,
                            )
                        nc.tensor.matmul(
                            pt[:],
                            S4[:, gt % 8, :],
                            G[:, cslot * 128:(cslot + 1) * 128],
                            start=(t == 0),
                            stop=(t == tw - 1),
                        )
                        th += 1
                        gt += 1
                    lo, hi = wi * 128, (wi + 1) * 128
                    if h == 0:
                        nc.vector.tensor_copy(acc[:, lo:hi], pt[:])
                    else:
                        nc.vector.tensor_add(acc[:, lo:hi], acc[:, lo:hi], pt[:])
                    last_touch = (h == 1) or (T_hi[wi] == 0)
                    if last_touch:
                        nc.sync.dma_start(out_d[lo:hi, :], acc[:, lo:hi])
    nc.compile()
    return nc


def kernel(x, edge_index):
    global LAST_RESULT
    _ensure_ntff_hook()
    from concourse.bass_utils import run_bass_kernel_spmd

    per_core, T_lo, T_hi, L_lo, L_hi = _host_prep(x, edge_index)

    key = (T_lo, T_hi)
    if key not in _prog_cache:
        _prog_cache[key] = _build_program(T_lo, T_hi, L_lo, L_hi)
    nc = _prog_cache[key]

    res = run_bass_kernel_spmd(nc, per_core, core_ids=list(range(C)))
    LAST_RESULT = res
    out = np.concatenate([r["out"][:NLOC] for r in res.results], axis=0)
    return out.astype(np.float32)

